# revision 2
# baseline (speedup 1.0000x reference)
"""BilateralFilter (SqueezeSeg mc condensing-kernel gaussians) on 8 TRN2 cores.

Reference computes, for x: [16, 64, 512, 3] (B, Z, A, C=xyz):
    nbr   = 14 spatial neighbors of each pixel in a 3x5 window (zero-padded)
    diff2 = sum_c (x - nbr)^2                           [B, Z, A, 14]
    out   = exp(-diff2 / (2 * theta_r^2))               [B, Z, A, 14, 4]
with THETA_R = [0.015, 0.015, 0.01, 0.01] (only 2 distinct values).

Strategy (pure batch data-parallel, 2 batches per core):
  - partitions p = b*64 + z  (128), free dim = azimuth chunks (AC wide).
  - squared differences via a runtime-registered fused custom DVE op
    (out = (in0-in1)^2), channel sums via tensor_reduce.
  - mirror symmetry: m_k(q) = |x(q) - x(q+off_k)|^2 for the 7 "negative"
    offsets k=0..6 gives the other 7 via diff2_{13-k}(q) = m_k(q - off_k);
    the z+1-partition read (engines cannot shift partitions by 1) is
    materialized on the idle TensorE as an exact 0/1 permutation matmul
    into PSUM, with the phantom z=64 boundary row (out-of-image neighbor
    => diff2 = |x(center)|^2, from s = sum_c x^2) accumulated by a second
    selector matmul. (PE_SHIFT=False falls back to partition-remap DMAs.)
  - ACT computes exp with the free scale immediate; each exp is written to
    both classes of its theta pair via a stride-0 input axis, directly into
    the interleaved [a, k, c] staging layout.
  - the staging tile matches DRAM layout exactly, so the store is one
    contiguous 128-partition DMA (28 KB/partition runs at AC=128).
"""

import numpy as np

import concourse.bass as bass
import concourse.tile as tile
from concourse import bacc, mybir
from concourse.bass_utils import run_bass_kernel_spmd

N_CORES = 8
B, Z, A, C = 16, 64, 512, 3
K, NCLS = 14, 4
LB = B // N_CORES            # local batches per core = 2
P = LB * Z                   # 128 partitions
AC = 128                     # azimuth chunk
BUFS = 3                     # tile pool buffers
PE_SHIFT = True              # z+1 partition shift via PE matmul vs SBUF DMA
XDN_PE = False               # derive x_dn on PE too (serializes behind xt load)
F32 = mybir.dt.float32


def _host_shift_mats():
    """SH2[k, m] = 1 iff k == m+1 (and not m == 63: batch boundary);
    SEL[k, m] = 1 iff k == m in {63, 127} (phantom z=64 row selector);
    SHD[k, m] = 1 iff k == m-1 (and not m in {0, 64}: z=0 rows stay 0)."""
    sh = np.zeros((P, P), np.float32)
    for m in range(P - 1):
        if m != Z - 1:
            sh[m + 1, m] = 1.0
    sel = np.zeros((P, P), np.float32)
    sel[Z - 1, Z - 1] = 1.0
    sel[P - 1, P - 1] = 1.0
    shd = np.zeros((P, P), np.float32)
    for m in range(1, P):
        if m != Z:
            shd[m - 1, m] = 1.0
    return sh, sel, shd

# exp scales: -1 / (2 * theta^2), theta pairs (0.015, 0.01), f32 semantics
_t0 = np.float32(0.015)
_t1 = np.float32(0.01)
SC0 = -float(1.0 / np.float32(np.float32(2.0) * _t0 * _t0))
SC1 = -float(1.0 / np.float32(np.float32(2.0) * _t1 * _t1))

# DRAM strides (elements) of out [LB, Z, A, K, NCLS]
O_A = K * NCLS               # 56
O_Z = A * O_A                # 28672
O_B = Z * O_Z                # 1835008
X_Z = A * C                  # 1536
X_B = Z * X_Z


def _ap(t, poff, pcnt, foff, pairs, pstep=1):
    """AP on tile t: partitions [poff, poff+pcnt) (stride pstep rows), free
    `pairs` ([step, count] in elements) based at element foff."""
    row = t.ap[0][0]
    return bass.AP(tensor=t.tensor, offset=t.offset + poff * row + foff,
                   ap=[[pstep * row, pcnt]] + [list(p) for p in pairs])


_SQDIFF = None


def _get_sqdiff():
    """Register a runtime custom DVE op: out = (in0 - in1)^2 (fp32, one
    instruction instead of subtract + multiply)."""
    global _SQDIFF
    if _SQDIFF is not None:
        return _SQDIFF
    from concourse import dve_ops
    from concourse.dve_spec import Spec, Src0, Src1, sq, lower, _has_src1
    from concourse.dve_uop import DveOpSpec

    name = "SQDIFF_BILAT_ANT"
    if name not in dve_ops._SUB_OPCODE_FOR_NAME:
        spec = Spec(
            body=sq(Src0 - Src1),
            reference=lambda in0, in1, c0, c1, c2:
                (in0.astype(np.float32) - in1.astype(np.float32)) ** 2)
        row = 1 + len(dve_ops.OPS)
        assert row < 0x20
        shas = {}
        for ver in ("v3",):
            tmp = DveOpSpec(name=name, opcode=row, uops=lower(spec, ver=ver),
                            rd1_en=_has_src1(spec))
            shas[ver] = tmp.sha(ver)
        op = dve_ops.DveOp(name, spec, subdim=False, uops_sha=shas)
        dve_ops.OPS.append(op)
        dve_ops.CUSTOM_DVE_SPECS[name] = spec
        dve_ops._SUB_OPCODE_FOR_NAME[name] = row
    else:
        op = next(o for o in dve_ops.OPS if o.name == name)
    _SQDIFF = op
    return op


def _build(ac=AC, bufs=BUFS, reps=1, pe_shift=PE_SHIFT, xdn_pe=XDN_PE,
           chunks=None, psum_bufs=3):
    # chunk schedule: list of (a0, width); smaller leading chunks shorten
    # the pipeline fill before the first store
    if chunks is None:
        chunks = [(0, 64), (64, 64)] + [(a0, ac) for a0 in range(128, A, ac)]
    assert sum(w for _, w in chunks) == A
    NCH = len(chunks)
    nc = bacc.Bacc("TRN2", target_bir_lowering=False, debug=False,
                   num_devices=N_CORES)
    x_h = nc.dram_tensor("x", [LB, Z, A, C], F32, kind="ExternalInput")
    o_h = nc.dram_tensor("out", [LB, Z, A, K, NCLS], F32, kind="ExternalOutput")
    x_ap, o_ap = x_h.ap(), o_h.ap()
    if pe_shift:
        shm_h = nc.dram_tensor("shm", [P, P], F32, kind="ExternalInput")
        sel_h = nc.dram_tensor("sel", [P, P], F32, kind="ExternalInput")
        if xdn_pe:
            shd_h = nc.dram_tensor("shd", [P, P], F32, kind="ExternalInput")
    # bench mode: reps > 1 re-runs the whole kernel; each non-final pass
    # stores to its own DRAM scratch so stores are real traffic
    scratch_aps = [
        nc.dram_tensor(f"scr{r}", [LB, Z, A, K, NCLS], F32).ap()
        for r in range(reps - 1)]

    from contextlib import ExitStack
    with tile.TileContext(nc) as tc, ExitStack() as es:
        if pe_shift:
            consts = es.enter_context(tc.tile_pool(name="consts", bufs=1))
            psum = es.enter_context(
                tc.tile_pool(name="psum", bufs=psum_bufs, space="PSUM"))
        with tc.tile_pool(name="pool", bufs=bufs) as pool:
            if pe_shift:
                sh_t = consts.tile([P, P], F32, name="sh_t")
                nc.sync.dma_start(sh_t[:], shm_h.ap()[:])
                sel_t = consts.tile([P, P], F32, name="sel_t")
                nc.sync.dma_start(sel_t[:], sel_h.ap()[:])
                if xdn_pe:
                    shd_t = consts.tile([P, P], F32, name="shd_t")
                    nc.sync.dma_start(shd_t[:], shd_h.ap()[:])
            for ci in range(NCH * reps):
                a0, ac = chunks[ci % NCH]
                XW = ac + 8          # x window (halo 4 each side)
                MW = ac + 4          # m window (halo 2 each side)
                lo, hi = max(0, a0 - 4), min(A, a0 + ac + 4)
                c_lo = (lo - (a0 - 4)) * C          # first valid xt col
                c_hi = (hi - (a0 - 4)) * C

                # ---- load x window (zero halo at image borders) ----
                # (b, z) rows are contiguous in DRAM: one 128-partition DMA
                xt = pool.tile([P, XW * C], F32, name="xt")
                if c_lo > 0:
                    nc.gpsimd.memset(_ap(xt, 0, P, 0, [[1, c_lo]]), 0.0)
                if c_hi < XW * C:
                    nc.gpsimd.memset(
                        _ap(xt, 0, P, c_hi, [[1, XW * C - c_hi]]), 0.0)
                nc.sync.dma_start(
                    _ap(xt, 0, P, c_lo, [[C, hi - lo], [1, C]]),
                    bass.AP(tensor=x_ap.tensor, offset=lo * C,
                            ap=[[X_Z, P], [C, hi - lo], [1, C]]))

                # ---- x_dn[p] = x at (z-1) (zeros at z=0 rows) ----
                if pe_shift and xdn_pe:
                    # exact PE permutation shift of xt into PSUM; the zero
                    # columns of SHD give the z=0 rows (and the xt halo gives
                    # the image-border zeros) for free
                    x_dn = psum.tile([P, XW * C], F32, name="x_dn_ps")
                    for n0 in range(0, XW * C, 512):
                        n1 = min(XW * C, n0 + 512)
                        nc.tensor.matmul(
                            _ap(x_dn, 0, P, n0, [[1, n1 - n0]]),
                            shd_t[:], _ap(xt, 0, P, n0, [[1, n1 - n0]]),
                            start=True, stop=True)
                else:
                    x_dn = pool.tile([P, XW * C], F32, name="x_dn")
                    nc.gpsimd.memset(x_dn[:], 0.0)
                    for b in range(LB):
                        nc.sync.dma_start(
                            _ap(x_dn, b * Z + 1, Z - 1, c_lo,
                                [[C, hi - lo], [1, C]]),
                            bass.AP(tensor=x_ap.tensor, offset=b * X_B + lo * C,
                                    ap=[[X_Z, Z - 1], [C, hi - lo], [1, C]]))

                # ---- s = sum_c x^2 over the full x window ----
                sqx = pool.tile([P, XW * C], F32, name="sqx")
                nc.scalar.square(sqx[:], xt[:])
                st = pool.tile([P, XW], F32, name="st")
                nc.vector.tensor_reduce(
                    st[:], _ap(sqx, 0, P, 0, [[C, XW], [1, C]]),
                    axis=mybir.AxisListType.X, op=mybir.AluOpType.add)

                # ---- m_k maps: M[p, k*MW + ar] over a-window [a0-2, a0+258)
                # k=0..4: dz=-1, da=k-2 ; k=5,6: dz=0, da=k-7
                # d2 = (x - x_nbr)^2 in one fused custom op per k
                sqdiff = _get_sqdiff()
                M = pool.tile([P, 7 * MW], F32, name="M")
                dt5 = pool.tile([P, 5 * MW * C], F32, name="dt5")
                for k in range(5):
                    nc.vector._custom_dve(
                        sqdiff,
                        out=_ap(dt5, 0, P, k * MW * C, [[C, MW], [1, C]]),
                        in0=_ap(xt, 0, P, 2 * C, [[C, MW], [1, C]]),
                        in1=_ap(x_dn, 0, P, k * C, [[C, MW], [1, C]]))
                nc.vector.tensor_reduce(
                    _ap(M, 0, P, 0, [[1, 5 * MW]]),
                    _ap(dt5, 0, P, 0, [[C, 5 * MW], [1, C]]),
                    axis=mybir.AxisListType.X, op=mybir.AluOpType.add)

                dt2 = pool.tile([P, 2 * MW * C], F32, name="dt2")
                for k in (5, 6):
                    nc.vector._custom_dve(
                        sqdiff,
                        out=_ap(dt2, 0, P, (k - 5) * MW * C, [[C, MW], [1, C]]),
                        in0=_ap(xt, 0, P, 2 * C, [[C, MW], [1, C]]),
                        in1=_ap(xt, 0, P, (k - 5) * C, [[C, MW], [1, C]]))
                nc.vector.tensor_reduce(
                    _ap(M, 0, P, 5 * MW, [[1, 2 * MW]]),
                    _ap(dt2, 0, P, 0, [[C, 2 * MW], [1, C]]),
                    axis=mybir.AxisListType.X, op=mybir.AluOpType.add)

                # ---- M_up[p] = M[p+1] for k=0..4 cols; phantom z=64 rows
                # ({63,127}) = s(z=63 row) with k-dependent a-shift ----
                if pe_shift:
                    # PE permutation matmul: M_up = SH2^T.T @ M + SEL.T @ SD
                    # (exact for 0/1 matrices); phantom rows ride the second
                    # accumulating matmul through SD
                    SD = pool.tile([P, 5 * MW], F32, name="SD")
                    nc.vector.tensor_copy(
                        _ap(SD, 0, P, 0, [[MW, 5], [1, MW]]),
                        _ap(st, 0, P, 0, [[1, 5], [1, MW]]))
                    M_up = psum.tile([P, 5 * MW], F32, name="M_up_ps")
                    for n0 in range(0, 5 * MW, 512):
                        n1 = min(5 * MW, n0 + 512)
                        nc.tensor.matmul(
                            _ap(M_up, 0, P, n0, [[1, n1 - n0]]),
                            sh_t[:], _ap(M, 0, P, n0, [[1, n1 - n0]]),
                            start=True, stop=False)
                        nc.tensor.matmul(
                            _ap(M_up, 0, P, n0, [[1, n1 - n0]]),
                            sel_t[:], _ap(SD, 0, P, n0, [[1, n1 - n0]]),
                            start=False, stop=True)
                else:
                    M_up = pool.tile([P, 5 * MW], F32, name="M_up")
                    # disjoint remaps per batch so the phantom DMA runs parallel
                    for b in range(LB):
                        nc.sync.dma_start(
                            _ap(M_up, b * Z, Z - 1, 0, [[1, 5 * MW]]),
                            _ap(M, b * Z + 1, Z - 1, 0, [[1, 5 * MW]]))
                    # phantom: M_up[{63,127}, k*MW + ar] = st[{63,127}, ar + k]
                    nc.sync.dma_start(
                        _ap(M_up, Z - 1, 2, 0, [[MW, 5], [1, MW]], pstep=Z),
                        _ap(st, Z - 1, 2, 0, [[1, 5], [1, MW]], pstep=Z))

                # ---- exps into O staging [p, ar*56 + k*4 + c] ----
                O = pool.tile([P, ac * O_A], F32, name="O",
                              bufs=(1 if ac >= 512 else None))
                for th, sc in ((0, SC0), (1, SC1)):
                    co = 2 * th
                    # direct k=0..6: in M[p, k*MW + ar + 2]
                    nc.scalar.activation(
                        _ap(O, 0, P, co, [[4, 7], [O_A, ac], [1, 2]]),
                        _ap(M, 0, P, 2, [[MW, 7], [1, ac], [0, 2]]),
                        mybir.ActivationFunctionType.Exp, scale=sc)
                    # a-mirrors k'=7,8 <- k=6,5: col = k*MW + ar + (9-k)
                    nc.scalar.activation(
                        _ap(O, 0, P, 28 + co, [[4, 2], [O_A, ac], [1, 2]]),
                        _ap(M, 0, P, 6 * MW + 3, [[-(MW - 1), 2], [1, ac], [0, 2]]),
                        mybir.ActivationFunctionType.Exp, scale=sc)
                    # dz-mirrors k'=9..13 <- k=4..0: M_up[p, k*MW + ar + 4 - k]
                    nc.scalar.activation(
                        _ap(O, 0, P, 36 + co, [[4, 5], [O_A, ac], [1, 2]]),
                        _ap(M_up, 0, P, 4 * (MW - 1) + 4,
                            [[-(MW - 1), 5], [1, ac], [0, 2]]),
                        mybir.ActivationFunctionType.Exp, scale=sc)

                # ---- store: one contiguous 128-partition DMA ----
                rep_i = ci // NCH
                dst_ap = o_ap if rep_i == reps - 1 else scratch_aps[rep_i]
                nc.sync.dma_start(
                    bass.AP(tensor=dst_ap.tensor, offset=a0 * O_A,
                            ap=[[O_Z, P], [1, ac * O_A]]),
                    _ap(O, 0, P, 0, [[1, ac * O_A]]))

    nc.compile()
    return nc


class _Runner:
    """Compile once; reuse the jitted sharded executable across calls.

    Mirrors bass2jax.run_bass_via_pjrt's multi-core path, but without
    donated output buffers (the kernel writes every output element, so the
    zero "output operands" are passed once from device-resident buffers and
    reused)."""

    def __init__(self, nc=None):
        import jax
        from jax.sharding import Mesh, PartitionSpec, NamedSharding
        try:
            from jax.experimental.shard_map import shard_map
        except ImportError:
            from jax.shard_map import shard_map  # newer jax
        from concourse import bass2jax

        bass2jax.install_neuronx_cc_hook()
        if nc is None:
            nc = _build()
        self.nc = nc

        partition_name = (nc.partition_id_tensor.name
                          if nc.partition_id_tensor else None)
        in_names, out_names, out_avals = [], [], []
        for alloc in nc.m.functions[0].allocations:
            if not isinstance(alloc, mybir.MemoryLocationSet):
                continue
            name = alloc.memorylocations[0].name
            if alloc.kind == "ExternalInput":
                if name != partition_name:
                    in_names.append(name)
            elif alloc.kind == "ExternalOutput":
                out_names.append(name)
                out_avals.append(jax.core.ShapedArray(
                    tuple(alloc.tensor_shape), mybir.dt.np(alloc.dtype)))
        assert set(in_names) <= {"x", "shm", "sel", "shd"}, in_names
        assert out_names == ["out"], out_names
        all_in_names = in_names + out_names
        if partition_name is not None:
            all_in_names = all_in_names + [partition_name]
        self.in_names = in_names

        def _body(*args):
            operands = list(args)
            if partition_name is not None:
                operands.append(bass2jax.partition_id_tensor())
            return tuple(bass2jax._bass_exec_p.bind(
                *operands,
                out_avals=tuple(out_avals),
                in_names=tuple(all_in_names),
                out_names=tuple(out_names),
                lowering_input_output_aliases=(),
                sim_require_finite=True,
                sim_require_nnan=True,
                nc=nc,
            ))

        devices = jax.devices()[:N_CORES]
        assert len(devices) == N_CORES
        self.mesh = Mesh(np.asarray(devices), ("core",))
        spec = PartitionSpec("core")
        rep = PartitionSpec()
        self.sharding = NamedSharding(self.mesh, spec)
        in_specs = tuple(spec if n == "x" else rep for n in in_names) + (spec,)
        self.jitted = jax.jit(shard_map(
            _body, mesh=self.mesh, in_specs=in_specs, out_specs=(spec,),
            check_rep=False))
        # device-resident constant operands, created once
        self.zeros_dev = jax.device_put(
            np.zeros((N_CORES * LB, Z, A, K, NCLS), np.float32), self.sharding)
        consts = {}
        if "shm" in in_names:
            shm, sel, shd = _host_shift_mats()
            rep_sh = NamedSharding(self.mesh, rep)
            consts["shm"] = jax.device_put(shm, rep_sh)
            consts["sel"] = jax.device_put(sel, rep_sh)
            consts["shd"] = jax.device_put(shd, rep_sh)
        self.consts = consts
        self._jax = jax

    def put(self, x: np.ndarray):
        return self._jax.device_put(
            np.ascontiguousarray(np.asarray(x, np.float32)), self.sharding)

    def run_dev(self, x_dev):
        """Execute; returns device array (not fetched)."""
        args = [x_dev if n == "x" else self.consts[n] for n in self.in_names]
        return self.jitted(*args, self.zeros_dev)[0]

    def __call__(self, x: np.ndarray) -> np.ndarray:
        return np.asarray(self.run_dev(self.put(x)))


_RUNNER = None


def _get_runner():
    global _RUNNER
    if _RUNNER is None:
        _RUNNER = _Runner()
    return _RUNNER


def kernel(x: np.ndarray) -> np.ndarray:
    x = np.asarray(x, dtype=np.float32)
    assert x.shape == (B, Z, A, C), x.shape
    try:
        return _get_runner()(x)
    except Exception:
        # fallback: reference-quality but slower dispatch path
        nc = _build()
        extra = {}
        if PE_SHIFT:
            shm, sel, shd = _host_shift_mats()
            extra = {"shm": shm, "sel": sel, "shd": shd}
        in_maps = [{"x": np.ascontiguousarray(x[i * LB:(i + 1) * LB]), **extra}
                   for i in range(N_CORES)]
        res = run_bass_kernel_spmd(nc, in_maps, list(range(N_CORES)))
        return np.concatenate(
            [res.results[i]["out"] for i in range(N_CORES)], axis=0)



# revision 8
# speedup vs baseline: 1.6716x; 1.6716x over previous
"""BilateralFilter (SqueezeSeg mc condensing-kernel gaussians) on 8 TRN2 cores.

Reference computes, for x: [16, 64, 512, 3] (B, Z, A, C=xyz):
    nbr   = 14 spatial neighbors of each pixel in a 3x5 window (zero-padded)
    diff2 = sum_c (x - nbr)^2                           [B, Z, A, 14]
    out   = exp(-diff2 / (2 * theta_r^2))               [B, Z, A, 14, 4]
with THETA_R = [0.015, 0.015, 0.01, 0.01] (only 2 distinct values).

Strategy (pure batch data-parallel, 2 batches per core):
  - partitions p = b*64 + z  (128), free dim = azimuth chunks (AC wide).
  - squared differences via a runtime-registered fused custom DVE op
    (out = (in0-in1)^2), channel sums via tensor_reduce.
  - mirror symmetry: m_k(q) = |x(q) - x(q+off_k)|^2 for the 7 "negative"
    offsets k=0..6 gives the other 7 via diff2_{13-k}(q) = m_k(q - off_k);
    the z+1-partition read (engines cannot shift partitions by 1) is
    materialized on the idle TensorE as an exact 0/1 permutation matmul
    into PSUM, with the phantom z=64 boundary row (out-of-image neighbor
    => diff2 = |x(center)|^2, from s = sum_c x^2) accumulated by a second
    selector matmul. (PE_SHIFT=False falls back to partition-remap DMAs.)
  - ACT computes exp with the free scale immediate; each exp is written to
    both classes of its theta pair via a stride-0 input axis, directly into
    the interleaved [a, k, c] staging layout.
  - the staging tile matches DRAM layout exactly, so the store is one
    contiguous 128-partition DMA (28 KB/partition runs at AC=128).
"""

import numpy as np

import concourse.bass as bass
import concourse.tile as tile
from concourse import bacc, mybir
from concourse.bass_utils import run_bass_kernel_spmd

N_CORES = 8
B, Z, A, C = 16, 64, 512, 3
K, NCLS = 14, 4
LB = B // N_CORES            # local batches per core = 2
P = LB * Z                   # 128 partitions
AC = 128                     # azimuth chunk
BUFS = 3                     # tile pool buffers
PE_SHIFT = True              # z+1 partition shift via PE matmul vs SBUF DMA
XDN_PE = True                # derive x_dn on PE too (no duplicate DRAM read)
F32 = mybir.dt.float32


def _host_shift_mats():
    """SH2[k, m] = 1 iff k == m+1 (and not m == 63: batch boundary);
    SEL[k, m] = 1 iff k == m in {63, 127} (phantom z=64 row selector);
    SHD[k, m] = 1 iff k == m-1 (and not m in {0, 64}: z=0 rows stay 0)."""
    sh = np.zeros((P, P), np.float32)
    for m in range(P - 1):
        if m != Z - 1:
            sh[m + 1, m] = 1.0
    sel = np.zeros((P, P), np.float32)
    sel[Z - 1, Z - 1] = 1.0
    sel[P - 1, P - 1] = 1.0
    shd = np.zeros((P, P), np.float32)
    for m in range(1, P):
        if m != Z:
            shd[m - 1, m] = 1.0
    return sh, sel, shd

# exp scales: -1 / (2 * theta^2), theta pairs (0.015, 0.01), f32 semantics
_t0 = np.float32(0.015)
_t1 = np.float32(0.01)
SC0 = -float(1.0 / np.float32(np.float32(2.0) * _t0 * _t0))
SC1 = -float(1.0 / np.float32(np.float32(2.0) * _t1 * _t1))

# DRAM strides (elements) of out [LB, Z, A, K, NCLS]
O_A = K * NCLS               # 56
O_Z = A * O_A                # 28672
O_B = Z * O_Z                # 1835008
X_Z = A * C                  # 1536
X_B = Z * X_Z


def _ap(t, poff, pcnt, foff, pairs, pstep=1):
    """AP on tile t: partitions [poff, poff+pcnt) (stride pstep rows), free
    `pairs` ([step, count] in elements) based at element foff."""
    row = t.ap[0][0]
    return bass.AP(tensor=t.tensor, offset=t.offset + poff * row + foff,
                   ap=[[pstep * row, pcnt]] + [list(p) for p in pairs])


_SQDIFF = None


def _get_sqdiff():
    """Register a runtime custom DVE op: out = (in0 - in1)^2 (fp32, one
    instruction instead of subtract + multiply)."""
    global _SQDIFF
    if _SQDIFF is not None:
        return _SQDIFF
    from concourse import dve_ops
    from concourse.dve_spec import Spec, Src0, Src1, sq, lower, _has_src1
    from concourse.dve_uop import DveOpSpec

    name = "SQDIFF_BILAT_ANT"
    if name not in dve_ops._SUB_OPCODE_FOR_NAME:
        spec = Spec(
            body=sq(Src0 - Src1),
            reference=lambda in0, in1, c0, c1, c2:
                (in0.astype(np.float32) - in1.astype(np.float32)) ** 2)
        row = 1 + len(dve_ops.OPS)
        assert row < 0x20
        shas = {}
        for ver in ("v3",):
            tmp = DveOpSpec(name=name, opcode=row, uops=lower(spec, ver=ver),
                            rd1_en=_has_src1(spec))
            shas[ver] = tmp.sha(ver)
        op = dve_ops.DveOp(name, spec, subdim=False, uops_sha=shas)
        dve_ops.OPS.append(op)
        dve_ops.CUSTOM_DVE_SPECS[name] = spec
        dve_ops._SUB_OPCODE_FOR_NAME[name] = row
    else:
        op = next(o for o in dve_ops.OPS if o.name == name)
    _SQDIFF = op
    return op


def _build(ac=AC, bufs=BUFS, reps=1, pe_shift=PE_SHIFT, xdn_pe=XDN_PE,
           chunks=None, psum_bufs=3):
    # chunk schedule: list of (a0, width); smaller leading chunks shorten
    # the pipeline fill before the first store
    if chunks is None:
        chunks = [(0, 64), (64, 64)] + [(a0, ac) for a0 in range(128, A, ac)]
    assert sum(w for _, w in chunks) == A
    NCH = len(chunks)
    nc = bacc.Bacc("TRN2", target_bir_lowering=False, debug=False,
                   num_devices=N_CORES)
    x_h = nc.dram_tensor("x", [LB, Z, A, C], F32, kind="ExternalInput")
    o_h = nc.dram_tensor("out", [LB, Z, A, K, NCLS], F32, kind="ExternalOutput")
    x_ap, o_ap = x_h.ap(), o_h.ap()
    if pe_shift:
        shm_h = nc.dram_tensor("shm", [P, P], F32, kind="ExternalInput")
        sel_h = nc.dram_tensor("sel", [P, P], F32, kind="ExternalInput")
        if xdn_pe:
            shd_h = nc.dram_tensor("shd", [P, P], F32, kind="ExternalInput")
    # bench mode: reps > 1 re-runs the whole kernel; each non-final pass
    # stores to its own DRAM scratch so stores are real traffic
    scratch_aps = [
        nc.dram_tensor(f"scr{r}", [LB, Z, A, K, NCLS], F32).ap()
        for r in range(reps - 1)]

    from contextlib import ExitStack
    with tile.TileContext(nc) as tc, ExitStack() as es:
        if pe_shift:
            consts = es.enter_context(tc.tile_pool(name="consts", bufs=1))
            psum = es.enter_context(
                tc.tile_pool(name="psum", bufs=psum_bufs, space="PSUM"))
        with tc.tile_pool(name="pool", bufs=bufs) as pool:
            if pe_shift:
                sh_t = consts.tile([P, P], F32, name="sh_t")
                nc.sync.dma_start(sh_t[:], shm_h.ap()[:])
                sel_t = consts.tile([P, P], F32, name="sel_t")
                nc.sync.dma_start(sel_t[:], sel_h.ap()[:])
                if xdn_pe:
                    shd_t = consts.tile([P, P], F32, name="shd_t")
                    nc.sync.dma_start(shd_t[:], shd_h.ap()[:])
            N = NCH * reps

            def _geom(ci):
                a0, ac = chunks[ci % NCH]
                XW = ac + 8          # x window (halo 4 each side)
                lo, hi = max(0, a0 - 4), min(A, a0 + ac + 4)
                c_lo = (lo - (a0 - 4)) * C          # first valid xt col
                c_hi = (hi - (a0 - 4)) * C
                return a0, ac, XW, lo, hi, c_lo, c_hi

            def emit_load(ci):
                # ---- load x window (zero halo at image borders) ----
                # (b, z) rows are contiguous in DRAM: one 128-partition DMA.
                # Loads issue on the (otherwise idle) gpsimd SWDGE so they
                # are not program-ordered behind the big store issues on SP
                # — the next chunks' loads must cut ahead of queued stores
                # or compute stalls behind them.
                # deep rotation: loads must be queued well before the big
                # stores they contend with, or they wait out a full 10 us
                # store before landing (xt is tiny: 1.6 KB/partition/buf)
                _, _, XW, lo, hi, c_lo, c_hi = _geom(ci)
                xt = pool.tile([P, XW * C], F32, name="xt", bufs=8)
                if c_lo > 0:
                    nc.gpsimd.memset(_ap(xt, 0, P, 0, [[1, c_lo]]), 0.0)
                if c_hi < XW * C:
                    nc.gpsimd.memset(
                        _ap(xt, 0, P, c_hi, [[1, XW * C - c_hi]]), 0.0)
                nc.gpsimd.dma_start(
                    _ap(xt, 0, P, c_lo, [[C, hi - lo], [1, C]]),
                    bass.AP(tensor=x_ap.tensor, offset=lo * C,
                            ap=[[X_Z, P], [C, hi - lo], [1, C]]))
                return xt

            def emit_xdn(ci, xt):
                # ---- x_dn[p] = x at (z-1) (zeros at z=0 rows): exact PE
                # permutation shift of xt into PSUM; the zero columns of SHD
                # give the z=0 rows (and the xt halo the image-border zeros)
                # for free.  Emitted one chunk AHEAD of the consuming chunk:
                # the PE is in-order, so x_dn(i+1) must precede M_up(i) or
                # the serial loop DVE(i) -> M_up(i) -> x_dn(i+1) -> DVE(i+1)
                # paces the pipeline above the store rate.  bufs=2 so the
                # psum pool fits 8 banks (M_up 3x2 + x_dn 2x1).
                _, _, XW, _, _, _, _ = _geom(ci)
                x_dn = psum.tile([P, XW * C], F32, name="x_dn_ps", bufs=2)
                for n0 in range(0, XW * C, 512):
                    n1 = min(XW * C, n0 + 512)
                    nc.tensor.matmul(
                        _ap(x_dn, 0, P, n0, [[1, n1 - n0]]),
                        shd_t[:], _ap(xt, 0, P, n0, [[1, n1 - n0]]),
                        start=True, stop=True)
                return x_dn

            PF = 7               # load prefetch distance (chunks ahead)
            xts, xdns = {}, {}
            for j in range(min(PF, N)):
                xts[j] = emit_load(j)
            if pe_shift and xdn_pe:
                xdns[0] = emit_xdn(0, xts[0])

            for ci in range(N):
                a0, ac, XW, lo, hi, c_lo, c_hi = _geom(ci)
                MW = ac + 4          # m window (halo 2 each side)

                if ci + PF < N:
                    xts[ci + PF] = emit_load(ci + PF)
                if pe_shift and xdn_pe and ci + 1 < N:
                    xdns[ci + 1] = emit_xdn(ci + 1, xts[ci + 1])
                xt = xts.pop(ci)

                if pe_shift and xdn_pe:
                    x_dn = xdns.pop(ci)
                else:
                    x_dn = pool.tile([P, XW * C], F32, name="x_dn")
                    nc.gpsimd.memset(x_dn[:], 0.0)
                    for b in range(LB):
                        nc.gpsimd.dma_start(
                            _ap(x_dn, b * Z + 1, Z - 1, c_lo,
                                [[C, hi - lo], [1, C]]),
                            bass.AP(tensor=x_ap.tensor, offset=b * X_B + lo * C,
                                    ap=[[X_Z, Z - 1], [C, hi - lo], [1, C]]))

                # ---- s = sum_c x^2 over the full x window ----
                sqx = pool.tile([P, XW * C], F32, name="sqx")
                nc.scalar.square(sqx[:], xt[:])
                st = pool.tile([P, XW], F32, name="st")
                nc.vector.tensor_reduce(
                    st[:], _ap(sqx, 0, P, 0, [[C, XW], [1, C]]),
                    axis=mybir.AxisListType.X, op=mybir.AluOpType.add)

                # ---- m_k maps: M[p, k*MW + ar] over a-window [a0-2, a0+258)
                # k=0..4: dz=-1, da=k-2 ; k=5,6: dz=0, da=k-7
                # d2 = (x - x_nbr)^2 in one fused custom op per k
                sqdiff = _get_sqdiff()
                M = pool.tile([P, 7 * MW], F32, name="M")
                dt5 = pool.tile([P, 5 * MW * C], F32, name="dt5")
                for k in range(5):
                    nc.vector._custom_dve(
                        sqdiff,
                        out=_ap(dt5, 0, P, k * MW * C, [[C, MW], [1, C]]),
                        in0=_ap(xt, 0, P, 2 * C, [[C, MW], [1, C]]),
                        in1=_ap(x_dn, 0, P, k * C, [[C, MW], [1, C]]))
                nc.vector.tensor_reduce(
                    _ap(M, 0, P, 0, [[1, 5 * MW]]),
                    _ap(dt5, 0, P, 0, [[C, 5 * MW], [1, C]]),
                    axis=mybir.AxisListType.X, op=mybir.AluOpType.add)

                dt2 = pool.tile([P, 2 * MW * C], F32, name="dt2")
                for k in (5, 6):
                    nc.vector._custom_dve(
                        sqdiff,
                        out=_ap(dt2, 0, P, (k - 5) * MW * C, [[C, MW], [1, C]]),
                        in0=_ap(xt, 0, P, 2 * C, [[C, MW], [1, C]]),
                        in1=_ap(xt, 0, P, (k - 5) * C, [[C, MW], [1, C]]))
                nc.vector.tensor_reduce(
                    _ap(M, 0, P, 5 * MW, [[1, 2 * MW]]),
                    _ap(dt2, 0, P, 0, [[C, 2 * MW], [1, C]]),
                    axis=mybir.AxisListType.X, op=mybir.AluOpType.add)

                # ---- M_up[p] = M[p+1] for k=0..4 cols; phantom z=64 rows
                # ({63,127}) = s(z=63 row) with k-dependent a-shift ----
                if pe_shift:
                    # PE permutation matmul: M_up = SH2^T.T @ M + SEL.T @ SD
                    # (exact for 0/1 matrices); phantom rows ride the second
                    # accumulating matmul through SD
                    SD = pool.tile([P, 5 * MW], F32, name="SD")
                    nc.vector.tensor_copy(
                        _ap(SD, 0, P, 0, [[MW, 5], [1, MW]]),
                        _ap(st, 0, P, 0, [[1, 5], [1, MW]]))
                    M_up = psum.tile([P, 5 * MW], F32, name="M_up_ps")
                    for n0 in range(0, 5 * MW, 512):
                        n1 = min(5 * MW, n0 + 512)
                        nc.tensor.matmul(
                            _ap(M_up, 0, P, n0, [[1, n1 - n0]]),
                            sh_t[:], _ap(M, 0, P, n0, [[1, n1 - n0]]),
                            start=True, stop=False)
                        nc.tensor.matmul(
                            _ap(M_up, 0, P, n0, [[1, n1 - n0]]),
                            sel_t[:], _ap(SD, 0, P, n0, [[1, n1 - n0]]),
                            start=False, stop=True)
                else:
                    M_up = pool.tile([P, 5 * MW], F32, name="M_up")
                    # disjoint remaps per batch so the phantom DMA runs parallel
                    for b in range(LB):
                        nc.sync.dma_start(
                            _ap(M_up, b * Z, Z - 1, 0, [[1, 5 * MW]]),
                            _ap(M, b * Z + 1, Z - 1, 0, [[1, 5 * MW]]))
                    # phantom: M_up[{63,127}, k*MW + ar] = st[{63,127}, ar + k]
                    nc.sync.dma_start(
                        _ap(M_up, Z - 1, 2, 0, [[MW, 5], [1, MW]], pstep=Z),
                        _ap(st, Z - 1, 2, 0, [[1, 5], [1, MW]], pstep=Z))

                # ---- exps into O staging [p, ar*56 + k*4 + c] ----
                O = pool.tile([P, ac * O_A], F32, name="O",
                              bufs=(1 if ac >= 512 else None))
                for th, sc in ((0, SC0), (1, SC1)):
                    co = 2 * th
                    # direct k=0..6: in M[p, k*MW + ar + 2]
                    nc.scalar.activation(
                        _ap(O, 0, P, co, [[4, 7], [O_A, ac], [1, 2]]),
                        _ap(M, 0, P, 2, [[MW, 7], [1, ac], [0, 2]]),
                        mybir.ActivationFunctionType.Exp, scale=sc)
                    # a-mirrors k'=7,8 <- k=6,5: col = k*MW + ar + (9-k)
                    nc.scalar.activation(
                        _ap(O, 0, P, 28 + co, [[4, 2], [O_A, ac], [1, 2]]),
                        _ap(M, 0, P, 6 * MW + 3, [[-(MW - 1), 2], [1, ac], [0, 2]]),
                        mybir.ActivationFunctionType.Exp, scale=sc)
                    # dz-mirrors k'=9..13 <- k=4..0: M_up[p, k*MW + ar + 4 - k]
                    nc.scalar.activation(
                        _ap(O, 0, P, 36 + co, [[4, 5], [O_A, ac], [1, 2]]),
                        _ap(M_up, 0, P, 4 * (MW - 1) + 4,
                            [[-(MW - 1), 5], [1, ac], [0, 2]]),
                        mybir.ActivationFunctionType.Exp, scale=sc)

                # ---- store: one contiguous 128-partition DMA ----
                rep_i = ci // NCH
                dst_ap = o_ap if rep_i == reps - 1 else scratch_aps[rep_i]
                nc.sync.dma_start(
                    bass.AP(tensor=dst_ap.tensor, offset=a0 * O_A,
                            ap=[[O_Z, P], [1, ac * O_A]]),
                    _ap(O, 0, P, 0, [[1, ac * O_A]]))

    nc.compile()
    return nc


class _Runner:
    """Compile once; reuse the jitted sharded executable across calls.

    Mirrors bass2jax.run_bass_via_pjrt's multi-core path, but without
    donated output buffers (the kernel writes every output element, so the
    zero "output operands" are passed once from device-resident buffers and
    reused)."""

    def __init__(self, nc=None):
        import jax
        from jax.sharding import Mesh, PartitionSpec, NamedSharding
        try:
            from jax.experimental.shard_map import shard_map
        except ImportError:
            from jax.shard_map import shard_map  # newer jax
        from concourse import bass2jax

        bass2jax.install_neuronx_cc_hook()
        if nc is None:
            nc = _build()
        self.nc = nc

        partition_name = (nc.partition_id_tensor.name
                          if nc.partition_id_tensor else None)
        in_names, out_names, out_avals = [], [], []
        for alloc in nc.m.functions[0].allocations:
            if not isinstance(alloc, mybir.MemoryLocationSet):
                continue
            name = alloc.memorylocations[0].name
            if alloc.kind == "ExternalInput":
                if name != partition_name:
                    in_names.append(name)
            elif alloc.kind == "ExternalOutput":
                out_names.append(name)
                out_avals.append(jax.core.ShapedArray(
                    tuple(alloc.tensor_shape), mybir.dt.np(alloc.dtype)))
        assert set(in_names) <= {"x", "shm", "sel", "shd"}, in_names
        assert out_names == ["out"], out_names
        all_in_names = in_names + out_names
        if partition_name is not None:
            all_in_names = all_in_names + [partition_name]
        self.in_names = in_names

        def _body(*args):
            operands = list(args)
            if partition_name is not None:
                operands.append(bass2jax.partition_id_tensor())
            return tuple(bass2jax._bass_exec_p.bind(
                *operands,
                out_avals=tuple(out_avals),
                in_names=tuple(all_in_names),
                out_names=tuple(out_names),
                lowering_input_output_aliases=(),
                sim_require_finite=True,
                sim_require_nnan=True,
                nc=nc,
            ))

        devices = jax.devices()[:N_CORES]
        assert len(devices) == N_CORES
        self.mesh = Mesh(np.asarray(devices), ("core",))
        spec = PartitionSpec("core")
        rep = PartitionSpec()
        self.sharding = NamedSharding(self.mesh, spec)
        in_specs = tuple(spec if n == "x" else rep for n in in_names) + (spec,)
        self.jitted = jax.jit(shard_map(
            _body, mesh=self.mesh, in_specs=in_specs, out_specs=(spec,),
            check_rep=False))
        # device-resident constant operands, created once
        self.zeros_dev = jax.device_put(
            np.zeros((N_CORES * LB, Z, A, K, NCLS), np.float32), self.sharding)
        consts = {}
        if "shm" in in_names:
            shm, sel, shd = _host_shift_mats()
            rep_sh = NamedSharding(self.mesh, rep)
            consts["shm"] = jax.device_put(shm, rep_sh)
            consts["sel"] = jax.device_put(sel, rep_sh)
            consts["shd"] = jax.device_put(shd, rep_sh)
        self.consts = consts
        self._jax = jax

    def put(self, x: np.ndarray):
        return self._jax.device_put(
            np.ascontiguousarray(np.asarray(x, np.float32)), self.sharding)

    def run_dev(self, x_dev):
        """Execute; returns device array (not fetched)."""
        args = [x_dev if n == "x" else self.consts[n] for n in self.in_names]
        return self.jitted(*args, self.zeros_dev)[0]

    def __call__(self, x: np.ndarray) -> np.ndarray:
        return np.asarray(self.run_dev(self.put(x)))


_RUNNER = None


def _get_runner():
    global _RUNNER
    if _RUNNER is None:
        _RUNNER = _Runner()
    return _RUNNER


def kernel(x: np.ndarray) -> np.ndarray:
    x = np.asarray(x, dtype=np.float32)
    assert x.shape == (B, Z, A, C), x.shape
    try:
        return _get_runner()(x)
    except Exception:
        # fallback: reference-quality but slower dispatch path
        nc = _build()
        extra = {}
        if PE_SHIFT:
            shm, sel, shd = _host_shift_mats()
            extra = {"shm": shm, "sel": sel, "shd": shd}
        in_maps = [{"x": np.ascontiguousarray(x[i * LB:(i + 1) * LB]), **extra}
                   for i in range(N_CORES)]
        res = run_bass_kernel_spmd(nc, in_maps, list(range(N_CORES)))
        return np.concatenate(
            [res.results[i]["out"] for i in range(N_CORES)], axis=0)



# revision 33
# speedup vs baseline: 2.3050x; 1.3790x over previous
"""BilateralFilter (SqueezeSeg mc condensing-kernel gaussians) on 8 TRN2 cores.

Reference computes, for x: [16, 64, 512, 3] (B, Z, A, C=xyz):
    nbr   = 14 spatial neighbors of each pixel in a 3x5 window (zero-padded)
    diff2 = sum_c (x - nbr)^2                           [B, Z, A, 14]
    out   = exp(-diff2 / (2 * theta_r^2))               [B, Z, A, 14, 4]
with THETA_R = [0.015, 0.015, 0.01, 0.01] (only 2 distinct values).

Strategy (pure batch data-parallel, 2 batches per core):
  - partitions p = b*64 + z  (128), free dim = azimuth chunks (AC wide).
  - squared differences via a runtime-registered fused custom DVE op
    (out = (in0-in1)^2), channel sums via tensor_reduce.
  - mirror symmetry: m_k(q) = |x(q) - x(q+off_k)|^2 for the 7 "negative"
    offsets k=0..6 gives the other 7 via diff2_{13-k}(q) = m_k(q - off_k);
    the z+1-partition read (engines cannot shift partitions by 1) is
    materialized on the idle TensorE as an exact 0/1 permutation matmul
    into PSUM, with the phantom z=64 boundary row (out-of-image neighbor
    => diff2 = |x(center)|^2, from s = sum_c x^2) accumulated by a second
    selector matmul. (PE_SHIFT=False falls back to partition-remap DMAs.)
  - ACT computes exp with the free scale immediate; each exp is written to
    both classes of its theta pair via a stride-0 input axis, directly into
    the interleaved [a, k, c] staging layout.
  - the staging tile matches DRAM layout exactly, so the store is one
    contiguous 128-partition DMA (28 KB/partition runs at AC=128).
"""

import numpy as np

import concourse.bass as bass
import concourse.tile as tile
from concourse import bacc, mybir
from concourse.bass_utils import run_bass_kernel_spmd

N_CORES = 8
B, Z, A, C = 16, 64, 512, 3
K, NCLS = 14, 4
LB = B // N_CORES            # local batches per core = 2
P = LB * Z                   # 128 partitions
AC = 128                     # azimuth chunk
BUFS = 3                     # tile pool buffers
PE_SHIFT = True              # z+1 partition shift via PE matmul vs SBUF DMA
XDN_PE = True                # derive x_dn on PE too (no duplicate DRAM read)
F32 = mybir.dt.float32
BF16 = mybir.dt.bfloat16


def _host_shift_mats():
    """SH2[k, m] = 1 iff k == m+1 (and not m == 63: batch boundary);
    SEL[k, m] = 1 iff k == m in {63, 127} (phantom z=64 row selector);
    SHD[k, m] = 1 iff k == m-1 (and not m in {0, 64}: z=0 rows stay 0)."""
    sh = np.zeros((P, P), np.float32)
    for m in range(P - 1):
        if m != Z - 1:
            sh[m + 1, m] = 1.0
    sel = np.zeros((P, P), np.float32)
    sel[Z - 1, Z - 1] = 1.0
    sel[P - 1, P - 1] = 1.0
    shd = np.zeros((P, P), np.float32)
    for m in range(1, P):
        if m != Z:
            shd[m - 1, m] = 1.0
    return sh, sel, shd

# exp scales: -1 / (2 * theta^2), theta pairs (0.015, 0.01), f32 semantics
_t0 = np.float32(0.015)
_t1 = np.float32(0.01)
SC0 = -float(1.0 / np.float32(np.float32(2.0) * _t0 * _t0))
SC1 = -float(1.0 / np.float32(np.float32(2.0) * _t1 * _t1))

# DRAM strides (elements) of out [LB, Z, A, K, NCLS]
O_A = K * NCLS               # 56
O_Z = A * O_A                # 28672
O_B = Z * O_Z                # 1835008
X_Z = A * C                  # 1536
X_B = Z * X_Z


def _ap(t, poff, pcnt, foff, pairs, pstep=1):
    """AP on tile t: partitions [poff, poff+pcnt) (stride pstep rows), free
    `pairs` ([step, count] in elements) based at element foff."""
    row = t.ap[0][0]
    return bass.AP(tensor=t.tensor, offset=t.offset + poff * row + foff,
                   ap=[[pstep * row, pcnt]] + [list(p) for p in pairs])


_SQDIFF = None


def _get_sqdiff():
    """Register a runtime custom DVE op: out = (in0 - in1)^2 (fp32, one
    instruction instead of subtract + multiply)."""
    global _SQDIFF
    if _SQDIFF is not None:
        return _SQDIFF
    from concourse import dve_ops
    from concourse.dve_spec import Spec, Src0, Src1, sq, lower, _has_src1
    from concourse.dve_uop import DveOpSpec

    name = "SQDIFF_BILAT_ANT"
    if name not in dve_ops._SUB_OPCODE_FOR_NAME:
        spec = Spec(
            body=sq(Src0 - Src1),
            reference=lambda in0, in1, c0, c1, c2:
                (in0.astype(np.float32) - in1.astype(np.float32)) ** 2)
        row = 1 + len(dve_ops.OPS)
        assert row < 0x20
        shas = {}
        for ver in ("v3",):
            tmp = DveOpSpec(name=name, opcode=row, uops=lower(spec, ver=ver),
                            rd1_en=_has_src1(spec))
            shas[ver] = tmp.sha(ver)
        op = dve_ops.DveOp(name, spec, subdim=False, uops_sha=shas)
        dve_ops.OPS.append(op)
        dve_ops.CUSTOM_DVE_SPECS[name] = spec
        dve_ops._SUB_OPCODE_FOR_NAME[name] = row
    else:
        op = next(o for o in dve_ops.OPS if o.name == name)
    _SQDIFF = op
    return op


def _build(ac=AC, bufs=BUFS, reps=1, pe_shift=PE_SHIFT, xdn_pe=XDN_PE,
           chunks=None, psum_bufs=3, store_rings=2, dt_mode="planes",
           amir_dve=0, sq_dve=False):
    # chunk schedule: list of (a0, width); smaller leading chunks shorten
    # the pipeline fill before the first store
    if chunks is None:
        chunks = [(0, 64), (64, 64)] + [(a0, ac) for a0 in range(128, A, ac)]
    assert sum(w for _, w in chunks) == A
    NCH = len(chunks)
    nc = bacc.Bacc("TRN2", target_bir_lowering=False, debug=False,
                   num_devices=N_CORES)
    x_h = nc.dram_tensor("x", [LB, Z, A, C], F32, kind="ExternalInput")
    o_h = nc.dram_tensor("out", [LB, Z, A, K, NCLS], F32, kind="ExternalOutput")
    x_ap, o_ap = x_h.ap(), o_h.ap()
    if pe_shift:
        # 0/1 shift matrices are exact in bf16 (2x PE when M is bf16)
        CDT = F32 if dt_mode == "reduce" else BF16
        shm_h = nc.dram_tensor("shm", [P, P], CDT, kind="ExternalInput")
        sel_h = nc.dram_tensor("sel", [P, P], CDT, kind="ExternalInput")
        if xdn_pe:
            shd_h = nc.dram_tensor("shd", [P, P], F32, kind="ExternalInput")
    # bench mode: reps > 1 re-runs the whole kernel; each non-final pass
    # stores to its own DRAM scratch so stores are real traffic
    scratch_aps = [
        nc.dram_tensor(f"scr{r}", [LB, Z, A, K, NCLS], F32).ap()
        for r in range(reps - 1)]

    from contextlib import ExitStack
    with tile.TileContext(nc) as tc, ExitStack() as es:
        if pe_shift:
            consts = es.enter_context(tc.tile_pool(name="consts", bufs=1))
            psum = es.enter_context(
                tc.tile_pool(name="psum", bufs=psum_bufs, space="PSUM"))
        with tc.tile_pool(name="pool", bufs=bufs) as pool:
            if pe_shift:
                sh_t = consts.tile([P, P], CDT, name="sh_t")
                nc.sync.dma_start(sh_t[:], shm_h.ap()[:])
                sel_t = consts.tile([P, P], CDT, name="sel_t")
                nc.sync.dma_start(sel_t[:], sel_h.ap()[:])
                if xdn_pe:
                    shd_t = consts.tile([P, P], F32, name="shd_t")
                    nc.sync.dma_start(shd_t[:], shd_h.ap()[:])
            N = NCH * reps

            def _geom(ci):
                a0, ac = chunks[ci % NCH]
                XW = ac + 8          # x window (halo 4 each side)
                lo, hi = max(0, a0 - 4), min(A, a0 + ac + 4)
                c_lo = (lo - (a0 - 4)) * C          # first valid xt col
                c_hi = (hi - (a0 - 4)) * C
                return a0, ac, XW, lo, hi, c_lo, c_hi

            def emit_load(ci):
                # ---- load x window (zero halo at image borders) ----
                # (b, z) rows are contiguous in DRAM: one 128-partition DMA.
                # Loads issue on the (otherwise idle) gpsimd SWDGE so they
                # are not program-ordered behind the big store issues on SP
                # — the next chunks' loads must cut ahead of queued stores
                # or compute stalls behind them.
                # deep rotation: loads must be queued well before the big
                # stores they contend with, or they wait out a full 10 us
                # store before landing (xt is tiny: 1.6 KB/partition/buf)
                _, _, XW, lo, hi, c_lo, c_hi = _geom(ci)
                xt = pool.tile([P, XW * C], F32, name="xt", bufs=8)
                if c_lo > 0:
                    nc.gpsimd.memset(_ap(xt, 0, P, 0, [[1, c_lo]]), 0.0)
                if c_hi < XW * C:
                    nc.gpsimd.memset(
                        _ap(xt, 0, P, c_hi, [[1, XW * C - c_hi]]), 0.0)
                nc.gpsimd.dma_start(
                    _ap(xt, 0, P, c_lo, [[C, hi - lo], [1, C]]),
                    bass.AP(tensor=x_ap.tensor, offset=lo * C,
                            ap=[[X_Z, P], [C, hi - lo], [1, C]]))
                return xt

            def emit_xdn(ci, xt):
                # ---- x_dn[p] = x at (z-1) (zeros at z=0 rows): exact PE
                # permutation shift of xt into PSUM; the zero columns of SHD
                # give the z=0 rows (and the xt halo the image-border zeros)
                # for free.  Emitted one chunk AHEAD of the consuming chunk:
                # the PE is in-order, so x_dn(i+1) must precede M_up(i) or
                # the serial loop DVE(i) -> M_up(i) -> x_dn(i+1) -> DVE(i+1)
                # paces the pipeline above the store rate.  bufs=2 so the
                # psum pool fits 8 banks (M_up 3x2 + x_dn 2x1).
                _, _, XW, _, _, _, _ = _geom(ci)
                x_dn = psum.tile([P, XW * C], F32, name="x_dn_ps", bufs=2)
                for n0 in range(0, XW * C, 512):
                    n1 = min(XW * C, n0 + 512)
                    nc.tensor.matmul(
                        _ap(x_dn, 0, P, n0, [[1, n1 - n0]]),
                        shd_t[:], _ap(xt, 0, P, n0, [[1, n1 - n0]]),
                        start=True, stop=True)
                return x_dn

            PF = 7               # load prefetch distance (chunks ahead)
            xts, xdns = {}, {}
            for j in range(min(PF, N)):
                xts[j] = emit_load(j)
            if pe_shift and xdn_pe:
                xdns[0] = emit_xdn(0, xts[0])

            for ci in range(N):
                a0, ac, XW, lo, hi, c_lo, c_hi = _geom(ci)
                MW = ac + 4          # m window (halo 2 each side)

                if ci + PF < N:
                    xts[ci + PF] = emit_load(ci + PF)
                if pe_shift and xdn_pe and ci + 1 < N:
                    xdns[ci + 1] = emit_xdn(ci + 1, xts[ci + 1])
                xt = xts.pop(ci)

                if pe_shift and xdn_pe:
                    x_dn = xdns.pop(ci)
                else:
                    x_dn = pool.tile([P, XW * C], F32, name="x_dn")
                    nc.gpsimd.memset(x_dn[:], 0.0)
                    for b in range(LB):
                        nc.gpsimd.dma_start(
                            _ap(x_dn, b * Z + 1, Z - 1, c_lo,
                                [[C, hi - lo], [1, C]]),
                            bass.AP(tensor=x_ap.tensor, offset=b * X_B + lo * C,
                                    ap=[[X_Z, Z - 1], [C, hi - lo], [1, C]]))

                # ---- s = sum_c x^2 ; m_k maps over a-window [a0-2, ...)
                # k=0..4: dz=-1, da=k-2 ; k=5,6: dz=0, da=k-7
                # d2 = (x - x_nbr)^2 in one fused custom op per k.
                # dt_mode picks how the c-sum is done:
                #  "reduce": f32 interleaved + TensorReduce (no fast mode)
                #  "iadds":  bf16 interleaved (packed writes) + 2 stride-3
                #            tensor_adds — each add processes N/3 elements,
                #            beating the reduce's full-N stream
                #  "planes": custom writes c-outer packed bf16 planes; the
                #            adds are fully packed and hit the DVE 2x mode
                # bf16 rounds only d^2 / s / M (<=0.4% rel => ~2e-3 max abs
                # on the exp output, far inside the 2e-2 tolerance).
                sqdiff = _get_sqdiff()
                MDT = F32 if dt_mode == "reduce" else BF16
                M = pool.tile([P, 7 * MW], MDT, name="M")
                if dt_mode == "reduce":
                    sqx = pool.tile([P, XW * C], F32, name="sqx")
                    nc.scalar.square(sqx[:], xt[:])
                    st = pool.tile([P, XW], F32, name="st")
                    nc.vector.tensor_reduce(
                        st[:], _ap(sqx, 0, P, 0, [[C, XW], [1, C]]),
                        axis=mybir.AxisListType.X, op=mybir.AluOpType.add)
                    dt5 = pool.tile([P, 5 * MW * C], F32, name="dt5")
                    for k in range(5):
                        nc.vector._custom_dve(
                            sqdiff,
                            out=_ap(dt5, 0, P, k * MW * C, [[C, MW], [1, C]]),
                            in0=_ap(xt, 0, P, 2 * C, [[C, MW], [1, C]]),
                            in1=_ap(x_dn, 0, P, k * C, [[C, MW], [1, C]]))
                    nc.vector.tensor_reduce(
                        _ap(M, 0, P, 0, [[1, 5 * MW]]),
                        _ap(dt5, 0, P, 0, [[C, 5 * MW], [1, C]]),
                        axis=mybir.AxisListType.X, op=mybir.AluOpType.add)
                    dt2 = pool.tile([P, 2 * MW * C], F32, name="dt2")
                    for k in (5, 6):
                        nc.vector._custom_dve(
                            sqdiff,
                            out=_ap(dt2, 0, P, (k - 5) * MW * C,
                                    [[C, MW], [1, C]]),
                            in0=_ap(xt, 0, P, 2 * C, [[C, MW], [1, C]]),
                            in1=_ap(xt, 0, P, (k - 5) * C, [[C, MW], [1, C]]))
                    nc.vector.tensor_reduce(
                        _ap(M, 0, P, 5 * MW, [[1, 2 * MW]]),
                        _ap(dt2, 0, P, 0, [[C, 2 * MW], [1, C]]),
                        axis=mybir.AxisListType.X, op=mybir.AluOpType.add)
                else:
                    # custom-dve APs are rank<=3: one call per map k
                    dt = pool.tile([P, 3 * 7 * MW], BF16, name="dt")
                    if dt_mode == "planes":
                        # c-outer stream: strided f32 reads, PACKED bf16
                        # plane writes (scattered 2-byte writes would RMW)
                        d_out = lambda k: _ap(dt, 0, P, k * MW,
                                              [[7 * MW, C], [1, MW]])
                        d_in = lambda t, off: _ap(t, 0, P, off,
                                                  [[1, C], [C, MW]])
                        add_ap = lambda c: _ap(dt, 0, P, c * 7 * MW,
                                               [[1, 7 * MW]])
                    else:  # iadds: natural interleaved stream, packed writes
                        d_out = lambda k: _ap(dt, 0, P, 3 * k * MW,
                                              [[C, MW], [1, C]])
                        d_in = lambda t, off: _ap(t, 0, P, off,
                                                  [[C, MW], [1, C]])
                        add_ap = lambda c: _ap(dt, 0, P, c,
                                               [[C, 7 * MW]])
                    for k in range(7):
                        src, off = (x_dn, k * C) if k < 5 else (xt, (k - 5) * C)
                        nc.vector._custom_dve(
                            sqdiff, out=d_out(k),
                            in0=d_in(xt, 2 * C), in1=d_in(src, off))
                    dts = pool.tile([P, 7 * MW], BF16, name="dts")
                    nc.vector.tensor_add(dts[:], add_ap(0), add_ap(1))
                    nc.vector.tensor_add(M[:], dts[:], add_ap(2))

                    # s = sum_c x^2 via the same layout trick
                    sqx = pool.tile([P, 3 * XW], BF16, name="sqx")
                    if dt_mode == "planes":
                        sq_out = _ap(sqx, 0, P, 0, [[XW, C], [1, XW]])
                        sq_in = _ap(xt, 0, P, 0, [[1, C], [C, XW]])
                        s_ap = lambda c: _ap(sqx, 0, P, c * XW, [[1, XW]])
                    else:
                        sq_out, sq_in = sqx[:], xt[:]
                        s_ap = lambda c: _ap(sqx, 0, P, c, [[C, XW]])
                    if sq_dve:
                        nc.vector.tensor_mul(sq_out, sq_in, sq_in)
                    else:
                        nc.scalar.square(sq_out, sq_in)
                    stt = pool.tile([P, XW], BF16, name="stt")
                    st = pool.tile([P, XW], BF16, name="st")
                    nc.vector.tensor_add(stt[:], s_ap(0), s_ap(1))
                    nc.vector.tensor_add(st[:], stt[:], s_ap(2))

                # ---- M_up[p] = M[p+1] for k=0..4 cols; phantom z=64 rows
                # ({63,127}) = s(z=63 row) with k-dependent a-shift ----
                if pe_shift:
                    # PE permutation matmul: M_up = SH2^T.T @ M + SEL.T @ SD
                    # (exact for 0/1 matrices, also in bf16); phantom rows
                    # ride the second accumulating matmul through SD.  In
                    # planes mode everything is bf16 => 2x PE rate and a 4x
                    # TensorCopy for SD.
                    SD = pool.tile([P, 5 * MW], MDT, name="SD")
                    nc.vector.tensor_copy(
                        _ap(SD, 0, P, 0, [[MW, 5], [1, MW]]),
                        _ap(st, 0, P, 0, [[1, 5], [1, MW]]))
                    M_up = psum.tile([P, 5 * MW], F32, name="M_up_ps")
                    for n0 in range(0, 5 * MW, 512):
                        n1 = min(5 * MW, n0 + 512)
                        nc.tensor.matmul(
                            _ap(M_up, 0, P, n0, [[1, n1 - n0]]),
                            sh_t[:], _ap(M, 0, P, n0, [[1, n1 - n0]]),
                            start=True, stop=False)
                        nc.tensor.matmul(
                            _ap(M_up, 0, P, n0, [[1, n1 - n0]]),
                            sel_t[:], _ap(SD, 0, P, n0, [[1, n1 - n0]]),
                            start=False, stop=True)
                else:
                    M_up = pool.tile([P, 5 * MW], F32, name="M_up")
                    # disjoint remaps per batch so the phantom DMA runs parallel
                    for b in range(LB):
                        nc.sync.dma_start(
                            _ap(M_up, b * Z, Z - 1, 0, [[1, 5 * MW]]),
                            _ap(M, b * Z + 1, Z - 1, 0, [[1, 5 * MW]]))
                    # phantom: M_up[{63,127}, k*MW + ar] = st[{63,127}, ar + k]
                    nc.sync.dma_start(
                        _ap(M_up, Z - 1, 2, 0, [[MW, 5], [1, MW]], pstep=Z),
                        _ap(st, Z - 1, 2, 0, [[1, 5], [1, MW]], pstep=Z))

                # ---- exps into O staging [p, ar*56 + k*4 + c] ----
                # amir_dve: the a-mirror slots k'=7,8 duplicate the direct
                # k=6,5 exps at shifted a — a same-partition DVE copy
                # (rebalances element writes from the bottleneck ACT onto
                # DVE), with a 2-column ACT patch at the chunk edge where
                # the copy source falls outside this O tile.
                O = pool.tile([P, ac * O_A], F32, name="O",
                              bufs=(1 if ac >= 512 else None))
                EXP = mybir.ActivationFunctionType.Exp
                for th, sc in ((0, SC0), (1, SC1)):
                    co = 2 * th
                    # direct k=0..6: in M[p, k*MW + ar + 2]
                    nc.scalar.activation(
                        _ap(O, 0, P, co, [[4, 7], [O_A, ac], [1, 2]]),
                        _ap(M, 0, P, 2, [[MW, 7], [1, ac], [0, 2]]),
                        EXP, scale=sc)
                    if amir_dve:
                        # boundary patch: k'=7,8 at a in {ac-2, ac-1}
                        nc.scalar.activation(
                            _ap(O, 0, P, 28 + co + (ac - 2) * O_A,
                                [[4, 2], [O_A, 2], [1, 2]]),
                            _ap(M, 0, P, 6 * MW + 3 + (ac - 2),
                                [[-(MW - 1), 2], [1, 2], [0, 2]]),
                            EXP, scale=sc)
                    else:
                        # a-mirrors k'=7,8 <- k=6,5: col = k*MW + ar + (9-k)
                        nc.scalar.activation(
                            _ap(O, 0, P, 28 + co, [[4, 2], [O_A, ac], [1, 2]]),
                            _ap(M, 0, P, 6 * MW + 3,
                                [[-(MW - 1), 2], [1, ac], [0, 2]]),
                            EXP, scale=sc)
                    # dz-mirrors k'=9..13 <- k=4..0: M_up[p, k*MW + ar + 4-k]
                    # (partition-shifted values, not copyable within a lane)
                    nc.scalar.activation(
                        _ap(O, 0, P, 36 + co, [[4, 5], [O_A, ac], [1, 2]]),
                        _ap(M_up, 0, P, 4 * (MW - 1) + 4,
                            [[-(MW - 1), 5], [1, ac], [0, 2]]),
                        EXP, scale=sc)
                if amir_dve:
                    # O[a, 7+j, c] = O[a+1+j, 6-j, c] for a < ac-2, all c
                    nc.vector.tensor_copy(
                        _ap(O, 0, P, 28, [[4, 2], [O_A, ac - 2], [1, 4]]),
                        _ap(O, 0, P, 80, [[52, 2], [O_A, ac - 2], [1, 4]]))

                # ---- store: one contiguous 128-partition DMA.  Alternate
                # across DGE rings: one HWDGE ring sustains ~418 GB/s, the
                # SP+ACT pair ~800; store_rings=3 adds the gpsimd SWDGE. ----
                rep_i = ci // NCH
                dst_ap = o_ap if rep_i == reps - 1 else scratch_aps[rep_i]
                st_eng = (nc.sync, nc.scalar, nc.gpsimd)[ci % store_rings]
                st_eng.dma_start(
                    bass.AP(tensor=dst_ap.tensor, offset=a0 * O_A,
                            ap=[[O_Z, P], [1, ac * O_A]]),
                    _ap(O, 0, P, 0, [[1, ac * O_A]]))

    nc.compile()
    return nc


class _Runner:
    """Compile once; reuse the jitted sharded executable across calls.

    Mirrors bass2jax.run_bass_via_pjrt's multi-core path, but without
    donated output buffers (the kernel writes every output element, so the
    zero "output operands" are passed once from device-resident buffers and
    reused)."""

    def __init__(self, nc=None):
        import jax
        from jax.sharding import Mesh, PartitionSpec, NamedSharding
        try:
            from jax.experimental.shard_map import shard_map
        except ImportError:
            from jax.shard_map import shard_map  # newer jax
        from concourse import bass2jax

        bass2jax.install_neuronx_cc_hook()
        if nc is None:
            nc = _build()
        self.nc = nc

        partition_name = (nc.partition_id_tensor.name
                          if nc.partition_id_tensor else None)
        in_names, out_names, out_avals = [], [], []
        in_dtypes = {}
        for alloc in nc.m.functions[0].allocations:
            if not isinstance(alloc, mybir.MemoryLocationSet):
                continue
            name = alloc.memorylocations[0].name
            if alloc.kind == "ExternalInput":
                if name != partition_name:
                    in_names.append(name)
                    in_dtypes[name] = mybir.dt.np(alloc.dtype)
            elif alloc.kind == "ExternalOutput":
                out_names.append(name)
                out_avals.append(jax.core.ShapedArray(
                    tuple(alloc.tensor_shape), mybir.dt.np(alloc.dtype)))
        self.in_dtypes = in_dtypes
        assert set(in_names) <= {"x", "shm", "sel", "shd"}, in_names
        assert out_names == ["out"], out_names
        all_in_names = in_names + out_names
        if partition_name is not None:
            all_in_names = all_in_names + [partition_name]
        self.in_names = in_names

        def _body(*args):
            operands = list(args)
            if partition_name is not None:
                operands.append(bass2jax.partition_id_tensor())
            return tuple(bass2jax._bass_exec_p.bind(
                *operands,
                out_avals=tuple(out_avals),
                in_names=tuple(all_in_names),
                out_names=tuple(out_names),
                lowering_input_output_aliases=(),
                sim_require_finite=True,
                sim_require_nnan=True,
                nc=nc,
            ))

        devices = jax.devices()[:N_CORES]
        assert len(devices) == N_CORES
        self.mesh = Mesh(np.asarray(devices), ("core",))
        spec = PartitionSpec("core")
        rep = PartitionSpec()
        self.sharding = NamedSharding(self.mesh, spec)
        in_specs = tuple(spec if n == "x" else rep for n in in_names) + (spec,)
        self.jitted = jax.jit(shard_map(
            _body, mesh=self.mesh, in_specs=in_specs, out_specs=(spec,),
            check_rep=False))
        # device-resident constant operands, created once
        self.zeros_dev = jax.device_put(
            np.zeros((N_CORES * LB, Z, A, K, NCLS), np.float32), self.sharding)
        consts = {}
        if "shm" in in_names:
            shm, sel, shd = _host_shift_mats()
            rep_sh = NamedSharding(self.mesh, rep)
            for n, arr in (("shm", shm), ("sel", sel), ("shd", shd)):
                if n in in_names:
                    consts[n] = jax.device_put(
                        arr.astype(in_dtypes[n]), rep_sh)
        self.consts = consts
        self._jax = jax

    def put(self, x: np.ndarray):
        return self._jax.device_put(
            np.ascontiguousarray(np.asarray(x, np.float32)), self.sharding)

    def run_dev(self, x_dev):
        """Execute; returns device array (not fetched)."""
        args = [x_dev if n == "x" else self.consts[n] for n in self.in_names]
        return self.jitted(*args, self.zeros_dev)[0]

    def __call__(self, x: np.ndarray) -> np.ndarray:
        return np.asarray(self.run_dev(self.put(x)))


_RUNNER = None


def _get_runner():
    global _RUNNER
    if _RUNNER is None:
        _RUNNER = _Runner()
    return _RUNNER


def kernel(x: np.ndarray) -> np.ndarray:
    x = np.asarray(x, dtype=np.float32)
    assert x.shape == (B, Z, A, C), x.shape
    try:
        return _get_runner()(x)
    except Exception:
        # fallback: reference-quality but slower dispatch path
        nc = _build()
        extra = {}
        if PE_SHIFT:
            shm, sel, shd = _host_shift_mats()
            cdt = mybir.dt.np(BF16)  # matches _build(planes=True) default
            extra = {"shm": shm.astype(cdt), "sel": sel.astype(cdt),
                     "shd": shd}
        in_maps = [{"x": np.ascontiguousarray(x[i * LB:(i + 1) * LB]), **extra}
                   for i in range(N_CORES)]
        res = run_bass_kernel_spmd(nc, in_maps, list(range(N_CORES)))
        return np.concatenate(
            [res.results[i]["out"] for i in range(N_CORES)], axis=0)



# revision 36
# speedup vs baseline: 4.7228x; 2.0489x over previous
"""BilateralFilter (SqueezeSeg mc condensing-kernel gaussians) on 8 TRN2 cores.

Reference computes, for x: [16, 64, 512, 3] (B, Z, A, C=xyz):
    nbr   = 14 spatial neighbors of each pixel in a 3x5 window (zero-padded)
    diff2 = sum_c (x - nbr)^2                           [B, Z, A, 14]
    out   = exp(-diff2 / (2 * theta_r^2))               [B, Z, A, 14, 4]
with THETA_R = [0.015, 0.015, 0.01, 0.01] (only 2 distinct values).

Strategy (pure batch data-parallel, 2 batches per core):
  - partitions p = b*64 + z  (128), free dim = azimuth chunks (AC wide).
  - squared differences via a runtime-registered fused custom DVE op
    (out = (in0-in1)^2), channel sums via tensor_reduce.
  - mirror symmetry: m_k(q) = |x(q) - x(q+off_k)|^2 for the 7 "negative"
    offsets k=0..6 gives the other 7 via diff2_{13-k}(q) = m_k(q - off_k);
    the z+1-partition read (engines cannot shift partitions by 1) is
    materialized on the idle TensorE as an exact 0/1 permutation matmul
    into PSUM, with the phantom z=64 boundary row (out-of-image neighbor
    => diff2 = |x(center)|^2, from s = sum_c x^2) accumulated by a second
    selector matmul. (PE_SHIFT=False falls back to partition-remap DMAs.)
  - ACT computes exp with the free scale immediate; each exp is written to
    both classes of its theta pair via a stride-0 input axis, directly into
    the interleaved [a, k, c] staging layout.
  - the staging tile matches DRAM layout exactly, so the store is one
    contiguous 128-partition DMA (28 KB/partition runs at AC=128).
"""

import numpy as np

import concourse.bass as bass
import concourse.tile as tile
from concourse import bacc, mybir
from concourse.bass_utils import run_bass_kernel_spmd

N_CORES = 8
B, Z, A, C = 16, 64, 512, 3
K, NCLS = 14, 4
LB = B // N_CORES            # local batches per core = 2
P = LB * Z                   # 128 partitions
AC = 128                     # azimuth chunk
BUFS = 3                     # tile pool buffers
PE_SHIFT = True              # z+1 partition shift via PE matmul vs SBUF DMA
XDN_PE = True                # derive x_dn on PE too (no duplicate DRAM read)
F32 = mybir.dt.float32
BF16 = mybir.dt.bfloat16


def _host_shift_mats():
    """SH2[k, m] = 1 iff k == m+1 (and not m == 63: batch boundary);
    SEL[k, m] = 1 iff k == m in {63, 127} (phantom z=64 row selector);
    SHD[k, m] = 1 iff k == m-1 (and not m in {0, 64}: z=0 rows stay 0)."""
    sh = np.zeros((P, P), np.float32)
    for m in range(P - 1):
        if m != Z - 1:
            sh[m + 1, m] = 1.0
    sel = np.zeros((P, P), np.float32)
    sel[Z - 1, Z - 1] = 1.0
    sel[P - 1, P - 1] = 1.0
    shd = np.zeros((P, P), np.float32)
    for m in range(1, P):
        if m != Z:
            shd[m - 1, m] = 1.0
    return sh, sel, shd

# exp scales: -1 / (2 * theta^2), theta pairs (0.015, 0.01), f32 semantics
_t0 = np.float32(0.015)
_t1 = np.float32(0.01)
SC0 = -float(1.0 / np.float32(np.float32(2.0) * _t0 * _t0))
SC1 = -float(1.0 / np.float32(np.float32(2.0) * _t1 * _t1))

# DRAM strides (elements) of out [LB, Z, A, K, NCLS]
O_A = K * NCLS               # 56
O_Z = A * O_A                # 28672
O_B = Z * O_Z                # 1835008
X_Z = A * C                  # 1536
X_B = Z * X_Z


def _ap(t, poff, pcnt, foff, pairs, pstep=1):
    """AP on tile t: partitions [poff, poff+pcnt) (stride pstep rows), free
    `pairs` ([step, count] in elements) based at element foff."""
    row = t.ap[0][0]
    return bass.AP(tensor=t.tensor, offset=t.offset + poff * row + foff,
                   ap=[[pstep * row, pcnt]] + [list(p) for p in pairs])


_SQDIFF = None


def _get_sqdiff():
    """Register a runtime custom DVE op: out = (in0 - in1)^2 (fp32, one
    instruction instead of subtract + multiply)."""
    global _SQDIFF
    if _SQDIFF is not None:
        return _SQDIFF
    from concourse import dve_ops
    from concourse.dve_spec import Spec, Src0, Src1, sq, lower, _has_src1
    from concourse.dve_uop import DveOpSpec

    name = "SQDIFF_BILAT_ANT"
    if name not in dve_ops._SUB_OPCODE_FOR_NAME:
        spec = Spec(
            body=sq(Src0 - Src1),
            reference=lambda in0, in1, c0, c1, c2:
                (in0.astype(np.float32) - in1.astype(np.float32)) ** 2)
        row = 1 + len(dve_ops.OPS)
        assert row < 0x20
        shas = {}
        for ver in ("v3",):
            tmp = DveOpSpec(name=name, opcode=row, uops=lower(spec, ver=ver),
                            rd1_en=_has_src1(spec))
            shas[ver] = tmp.sha(ver)
        op = dve_ops.DveOp(name, spec, subdim=False, uops_sha=shas)
        dve_ops.OPS.append(op)
        dve_ops.CUSTOM_DVE_SPECS[name] = spec
        dve_ops._SUB_OPCODE_FOR_NAME[name] = row
    else:
        op = next(o for o in dve_ops.OPS if o.name == name)
    _SQDIFF = op
    return op


def _build(ac=AC, bufs=BUFS, reps=1, pe_shift=PE_SHIFT, xdn_pe=XDN_PE,
           chunks=None, psum_bufs=3, store_rings=2, dt_mode="planes",
           amir_dve=0, sq_dve=False):
    # chunk schedule: list of (a0, width).  Uniform chunks minimize the
    # per-chunk fixed instruction overhead (~185 ns per ACT instruction,
    # 6 of them per chunk); with deep load prefetch the pipeline fill no
    # longer needs smaller leading chunks, and fill amortizes over reps.
    if chunks is None:
        chunks = [(a0, ac) for a0 in range(0, A, ac)]
    assert sum(w for _, w in chunks) == A
    NCH = len(chunks)
    nc = bacc.Bacc("TRN2", target_bir_lowering=False, debug=False,
                   num_devices=N_CORES)
    x_h = nc.dram_tensor("x", [LB, Z, A, C], F32, kind="ExternalInput")
    o_h = nc.dram_tensor("out", [LB, Z, A, K, NCLS], F32, kind="ExternalOutput")
    x_ap, o_ap = x_h.ap(), o_h.ap()
    if pe_shift:
        # 0/1 shift matrices are exact in bf16 (2x PE when M is bf16)
        CDT = F32 if dt_mode == "reduce" else BF16
        shm_h = nc.dram_tensor("shm", [P, P], CDT, kind="ExternalInput")
        sel_h = nc.dram_tensor("sel", [P, P], CDT, kind="ExternalInput")
        if xdn_pe:
            shd_h = nc.dram_tensor("shd", [P, P], F32, kind="ExternalInput")
    # bench mode: reps > 1 re-runs the whole kernel; each non-final pass
    # stores to its own DRAM scratch so stores are real traffic
    scratch_aps = [
        nc.dram_tensor(f"scr{r}", [LB, Z, A, K, NCLS], F32).ap()
        for r in range(reps - 1)]

    from contextlib import ExitStack
    with tile.TileContext(nc) as tc, ExitStack() as es:
        if pe_shift:
            consts = es.enter_context(tc.tile_pool(name="consts", bufs=1))
            psum = es.enter_context(
                tc.tile_pool(name="psum", bufs=psum_bufs, space="PSUM"))
        with tc.tile_pool(name="pool", bufs=bufs) as pool:
            if pe_shift:
                sh_t = consts.tile([P, P], CDT, name="sh_t")
                nc.sync.dma_start(sh_t[:], shm_h.ap()[:])
                sel_t = consts.tile([P, P], CDT, name="sel_t")
                nc.sync.dma_start(sel_t[:], sel_h.ap()[:])
                if xdn_pe:
                    shd_t = consts.tile([P, P], F32, name="shd_t")
                    nc.sync.dma_start(shd_t[:], shd_h.ap()[:])
            N = NCH * reps

            def _geom(ci):
                a0, ac = chunks[ci % NCH]
                XW = ac + 8          # x window (halo 4 each side)
                lo, hi = max(0, a0 - 4), min(A, a0 + ac + 4)
                c_lo = (lo - (a0 - 4)) * C          # first valid xt col
                c_hi = (hi - (a0 - 4)) * C
                return a0, ac, XW, lo, hi, c_lo, c_hi

            def emit_load(ci):
                # ---- load x window (zero halo at image borders) ----
                # (b, z) rows are contiguous in DRAM: one 128-partition DMA.
                # Loads issue on the (otherwise idle) gpsimd SWDGE so they
                # are not program-ordered behind the big store issues on SP
                # — the next chunks' loads must cut ahead of queued stores
                # or compute stalls behind them.
                # deep rotation: loads must be queued well before the big
                # stores they contend with, or they wait out a full 10 us
                # store before landing (xt is tiny: 1.6 KB/partition/buf)
                _, _, XW, lo, hi, c_lo, c_hi = _geom(ci)
                xt = pool.tile([P, XW * C], F32, name="xt", bufs=8)
                if c_lo > 0:
                    nc.gpsimd.memset(_ap(xt, 0, P, 0, [[1, c_lo]]), 0.0)
                if c_hi < XW * C:
                    nc.gpsimd.memset(
                        _ap(xt, 0, P, c_hi, [[1, XW * C - c_hi]]), 0.0)
                nc.gpsimd.dma_start(
                    _ap(xt, 0, P, c_lo, [[C, hi - lo], [1, C]]),
                    bass.AP(tensor=x_ap.tensor, offset=lo * C,
                            ap=[[X_Z, P], [C, hi - lo], [1, C]]))
                return xt

            def emit_xdn(ci, xt):
                # ---- x_dn[p] = x at (z-1) (zeros at z=0 rows): exact PE
                # permutation shift of xt into PSUM; the zero columns of SHD
                # give the z=0 rows (and the xt halo the image-border zeros)
                # for free.  Emitted one chunk AHEAD of the consuming chunk:
                # the PE is in-order, so x_dn(i+1) must precede M_up(i) or
                # the serial loop DVE(i) -> M_up(i) -> x_dn(i+1) -> DVE(i+1)
                # paces the pipeline above the store rate.  bufs=2 so the
                # psum pool fits 8 banks (M_up 3x2 + x_dn 2x1).
                _, _, XW, _, _, _, _ = _geom(ci)
                x_dn = psum.tile([P, XW * C], F32, name="x_dn_ps", bufs=2)
                for n0 in range(0, XW * C, 512):
                    n1 = min(XW * C, n0 + 512)
                    nc.tensor.matmul(
                        _ap(x_dn, 0, P, n0, [[1, n1 - n0]]),
                        shd_t[:], _ap(xt, 0, P, n0, [[1, n1 - n0]]),
                        start=True, stop=True)
                return x_dn

            PF = 7               # load prefetch distance (chunks ahead)
            xts, xdns = {}, {}
            for j in range(min(PF, N)):
                xts[j] = emit_load(j)
            if pe_shift and xdn_pe:
                xdns[0] = emit_xdn(0, xts[0])

            for ci in range(N):
                a0, ac, XW, lo, hi, c_lo, c_hi = _geom(ci)
                MW = ac + 4          # m window (halo 2 each side)

                if ci + PF < N:
                    xts[ci + PF] = emit_load(ci + PF)
                if pe_shift and xdn_pe and ci + 1 < N:
                    xdns[ci + 1] = emit_xdn(ci + 1, xts[ci + 1])
                xt = xts.pop(ci)

                if pe_shift and xdn_pe:
                    x_dn = xdns.pop(ci)
                else:
                    x_dn = pool.tile([P, XW * C], F32, name="x_dn")
                    nc.gpsimd.memset(x_dn[:], 0.0)
                    for b in range(LB):
                        nc.gpsimd.dma_start(
                            _ap(x_dn, b * Z + 1, Z - 1, c_lo,
                                [[C, hi - lo], [1, C]]),
                            bass.AP(tensor=x_ap.tensor, offset=b * X_B + lo * C,
                                    ap=[[X_Z, Z - 1], [C, hi - lo], [1, C]]))

                # ---- s = sum_c x^2 ; m_k maps over a-window [a0-2, ...)
                # k=0..4: dz=-1, da=k-2 ; k=5,6: dz=0, da=k-7
                # d2 = (x - x_nbr)^2 in one fused custom op per k.
                # dt_mode picks how the c-sum is done:
                #  "reduce": f32 interleaved + TensorReduce (no fast mode)
                #  "iadds":  bf16 interleaved (packed writes) + 2 stride-3
                #            tensor_adds — each add processes N/3 elements,
                #            beating the reduce's full-N stream
                #  "planes": custom writes c-outer packed bf16 planes; the
                #            adds are fully packed and hit the DVE 2x mode
                # bf16 rounds only d^2 / s / M (<=0.4% rel => ~2e-3 max abs
                # on the exp output, far inside the 2e-2 tolerance).
                sqdiff = _get_sqdiff()
                MDT = F32 if dt_mode == "reduce" else BF16
                M = pool.tile([P, 7 * MW], MDT, name="M")
                if dt_mode == "reduce":
                    sqx = pool.tile([P, XW * C], F32, name="sqx")
                    nc.scalar.square(sqx[:], xt[:])
                    st = pool.tile([P, XW], F32, name="st")
                    nc.vector.tensor_reduce(
                        st[:], _ap(sqx, 0, P, 0, [[C, XW], [1, C]]),
                        axis=mybir.AxisListType.X, op=mybir.AluOpType.add)
                    dt5 = pool.tile([P, 5 * MW * C], F32, name="dt5")
                    for k in range(5):
                        nc.vector._custom_dve(
                            sqdiff,
                            out=_ap(dt5, 0, P, k * MW * C, [[C, MW], [1, C]]),
                            in0=_ap(xt, 0, P, 2 * C, [[C, MW], [1, C]]),
                            in1=_ap(x_dn, 0, P, k * C, [[C, MW], [1, C]]))
                    nc.vector.tensor_reduce(
                        _ap(M, 0, P, 0, [[1, 5 * MW]]),
                        _ap(dt5, 0, P, 0, [[C, 5 * MW], [1, C]]),
                        axis=mybir.AxisListType.X, op=mybir.AluOpType.add)
                    dt2 = pool.tile([P, 2 * MW * C], F32, name="dt2")
                    for k in (5, 6):
                        nc.vector._custom_dve(
                            sqdiff,
                            out=_ap(dt2, 0, P, (k - 5) * MW * C,
                                    [[C, MW], [1, C]]),
                            in0=_ap(xt, 0, P, 2 * C, [[C, MW], [1, C]]),
                            in1=_ap(xt, 0, P, (k - 5) * C, [[C, MW], [1, C]]))
                    nc.vector.tensor_reduce(
                        _ap(M, 0, P, 5 * MW, [[1, 2 * MW]]),
                        _ap(dt2, 0, P, 0, [[C, 2 * MW], [1, C]]),
                        axis=mybir.AxisListType.X, op=mybir.AluOpType.add)
                else:
                    # custom-dve APs are rank<=3: one call per map k
                    dt = pool.tile([P, 3 * 7 * MW], BF16, name="dt")
                    if dt_mode == "planes":
                        # c-outer stream: strided f32 reads, PACKED bf16
                        # plane writes (scattered 2-byte writes would RMW)
                        d_out = lambda k: _ap(dt, 0, P, k * MW,
                                              [[7 * MW, C], [1, MW]])
                        d_in = lambda t, off: _ap(t, 0, P, off,
                                                  [[1, C], [C, MW]])
                        add_ap = lambda c: _ap(dt, 0, P, c * 7 * MW,
                                               [[1, 7 * MW]])
                    else:  # iadds: natural interleaved stream, packed writes
                        d_out = lambda k: _ap(dt, 0, P, 3 * k * MW,
                                              [[C, MW], [1, C]])
                        d_in = lambda t, off: _ap(t, 0, P, off,
                                                  [[C, MW], [1, C]])
                        add_ap = lambda c: _ap(dt, 0, P, c,
                                               [[C, 7 * MW]])
                    for k in range(7):
                        src, off = (x_dn, k * C) if k < 5 else (xt, (k - 5) * C)
                        nc.vector._custom_dve(
                            sqdiff, out=d_out(k),
                            in0=d_in(xt, 2 * C), in1=d_in(src, off))
                    dts = pool.tile([P, 7 * MW], BF16, name="dts")
                    nc.vector.tensor_add(dts[:], add_ap(0), add_ap(1))
                    nc.vector.tensor_add(M[:], dts[:], add_ap(2))

                    # s = sum_c x^2 via the same layout trick
                    sqx = pool.tile([P, 3 * XW], BF16, name="sqx")
                    if dt_mode == "planes":
                        sq_out = _ap(sqx, 0, P, 0, [[XW, C], [1, XW]])
                        sq_in = _ap(xt, 0, P, 0, [[1, C], [C, XW]])
                        s_ap = lambda c: _ap(sqx, 0, P, c * XW, [[1, XW]])
                    else:
                        sq_out, sq_in = sqx[:], xt[:]
                        s_ap = lambda c: _ap(sqx, 0, P, c, [[C, XW]])
                    if sq_dve:
                        nc.vector.tensor_mul(sq_out, sq_in, sq_in)
                    else:
                        nc.scalar.square(sq_out, sq_in)
                    stt = pool.tile([P, XW], BF16, name="stt")
                    st = pool.tile([P, XW], BF16, name="st")
                    nc.vector.tensor_add(stt[:], s_ap(0), s_ap(1))
                    nc.vector.tensor_add(st[:], stt[:], s_ap(2))

                # ---- M_up[p] = M[p+1] for k=0..4 cols; phantom z=64 rows
                # ({63,127}) = s(z=63 row) with k-dependent a-shift ----
                if pe_shift:
                    # PE permutation matmul: M_up = SH2^T.T @ M + SEL.T @ SD
                    # (exact for 0/1 matrices, also in bf16); phantom rows
                    # ride the second accumulating matmul through SD.  In
                    # planes mode everything is bf16 => 2x PE rate and a 4x
                    # TensorCopy for SD.
                    SD = pool.tile([P, 5 * MW], MDT, name="SD")
                    nc.vector.tensor_copy(
                        _ap(SD, 0, P, 0, [[MW, 5], [1, MW]]),
                        _ap(st, 0, P, 0, [[1, 5], [1, MW]]))
                    M_up = psum.tile([P, 5 * MW], F32, name="M_up_ps")
                    for n0 in range(0, 5 * MW, 512):
                        n1 = min(5 * MW, n0 + 512)
                        nc.tensor.matmul(
                            _ap(M_up, 0, P, n0, [[1, n1 - n0]]),
                            sh_t[:], _ap(M, 0, P, n0, [[1, n1 - n0]]),
                            start=True, stop=False)
                        nc.tensor.matmul(
                            _ap(M_up, 0, P, n0, [[1, n1 - n0]]),
                            sel_t[:], _ap(SD, 0, P, n0, [[1, n1 - n0]]),
                            start=False, stop=True)
                else:
                    M_up = pool.tile([P, 5 * MW], F32, name="M_up")
                    # disjoint remaps per batch so the phantom DMA runs parallel
                    for b in range(LB):
                        nc.sync.dma_start(
                            _ap(M_up, b * Z, Z - 1, 0, [[1, 5 * MW]]),
                            _ap(M, b * Z + 1, Z - 1, 0, [[1, 5 * MW]]))
                    # phantom: M_up[{63,127}, k*MW + ar] = st[{63,127}, ar + k]
                    nc.sync.dma_start(
                        _ap(M_up, Z - 1, 2, 0, [[MW, 5], [1, MW]], pstep=Z),
                        _ap(st, Z - 1, 2, 0, [[1, 5], [1, MW]], pstep=Z))

                # ---- exps into O staging [p, ar*56 + k*4 + c] ----
                # amir_dve: the a-mirror slots k'=7,8 duplicate the direct
                # k=6,5 exps at shifted a — a same-partition DVE copy
                # (rebalances element writes from the bottleneck ACT onto
                # DVE), with a 2-column ACT patch at the chunk edge where
                # the copy source falls outside this O tile.
                O = pool.tile([P, ac * O_A], F32, name="O",
                              bufs=(1 if ac >= 512 else
                                    2 if ac >= 256 else None))
                EXP = mybir.ActivationFunctionType.Exp
                for th, sc in ((0, SC0), (1, SC1)):
                    co = 2 * th
                    # direct k=0..6: in M[p, k*MW + ar + 2]
                    nc.scalar.activation(
                        _ap(O, 0, P, co, [[4, 7], [O_A, ac], [1, 2]]),
                        _ap(M, 0, P, 2, [[MW, 7], [1, ac], [0, 2]]),
                        EXP, scale=sc)
                    if amir_dve:
                        # boundary patch: k'=7,8 at a in {ac-2, ac-1}
                        nc.scalar.activation(
                            _ap(O, 0, P, 28 + co + (ac - 2) * O_A,
                                [[4, 2], [O_A, 2], [1, 2]]),
                            _ap(M, 0, P, 6 * MW + 3 + (ac - 2),
                                [[-(MW - 1), 2], [1, 2], [0, 2]]),
                            EXP, scale=sc)
                    else:
                        # a-mirrors k'=7,8 <- k=6,5: col = k*MW + ar + (9-k)
                        nc.scalar.activation(
                            _ap(O, 0, P, 28 + co, [[4, 2], [O_A, ac], [1, 2]]),
                            _ap(M, 0, P, 6 * MW + 3,
                                [[-(MW - 1), 2], [1, ac], [0, 2]]),
                            EXP, scale=sc)
                    # dz-mirrors k'=9..13 <- k=4..0: M_up[p, k*MW + ar + 4-k]
                    # (partition-shifted values, not copyable within a lane)
                    nc.scalar.activation(
                        _ap(O, 0, P, 36 + co, [[4, 5], [O_A, ac], [1, 2]]),
                        _ap(M_up, 0, P, 4 * (MW - 1) + 4,
                            [[-(MW - 1), 5], [1, ac], [0, 2]]),
                        EXP, scale=sc)
                if amir_dve:
                    # O[a, 7+j, c] = O[a+1+j, 6-j, c] for a < ac-2, all c
                    nc.vector.tensor_copy(
                        _ap(O, 0, P, 28, [[4, 2], [O_A, ac - 2], [1, 4]]),
                        _ap(O, 0, P, 80, [[52, 2], [O_A, ac - 2], [1, 4]]))

                # ---- store: contiguous 128-partition DMAs.  Each HWDGE
                # ring sustains ~418 GB/s independently (SP + ACT pair
                # ~800 aggregate), so the store of EVERY chunk is split
                # into equal per-ring pieces — alternating whole chunks
                # leaves one ring with 62.5% of the bytes and that ring's
                # 21.9 us/pass becomes the kernel's floor.  store_rings=3
                # adds the gpsimd SWDGE ring. ----
                rep_i = ci // NCH
                dst_ap = o_ap if rep_i == reps - 1 else scratch_aps[rep_i]
                engs = (nc.sync, nc.scalar, nc.gpsimd)[:store_rings]
                edges = [r * ac // store_rings for r in range(store_rings + 1)]
                for r, eng in enumerate(engs):
                    w0, w1 = edges[r], edges[r + 1]
                    eng.dma_start(
                        bass.AP(tensor=dst_ap.tensor,
                                offset=(a0 + w0) * O_A,
                                ap=[[O_Z, P], [1, (w1 - w0) * O_A]]),
                        _ap(O, 0, P, w0 * O_A, [[1, (w1 - w0) * O_A]]))

    nc.compile()
    return nc


class _Runner:
    """Compile once; reuse the jitted sharded executable across calls.

    Mirrors bass2jax.run_bass_via_pjrt's multi-core path, but without
    donated output buffers (the kernel writes every output element, so the
    zero "output operands" are passed once from device-resident buffers and
    reused)."""

    def __init__(self, nc=None):
        import jax
        from jax.sharding import Mesh, PartitionSpec, NamedSharding
        try:
            from jax.experimental.shard_map import shard_map
        except ImportError:
            from jax.shard_map import shard_map  # newer jax
        from concourse import bass2jax

        bass2jax.install_neuronx_cc_hook()
        if nc is None:
            nc = _build()
        self.nc = nc

        partition_name = (nc.partition_id_tensor.name
                          if nc.partition_id_tensor else None)
        in_names, out_names, out_avals = [], [], []
        in_dtypes = {}
        for alloc in nc.m.functions[0].allocations:
            if not isinstance(alloc, mybir.MemoryLocationSet):
                continue
            name = alloc.memorylocations[0].name
            if alloc.kind == "ExternalInput":
                if name != partition_name:
                    in_names.append(name)
                    in_dtypes[name] = mybir.dt.np(alloc.dtype)
            elif alloc.kind == "ExternalOutput":
                out_names.append(name)
                out_avals.append(jax.core.ShapedArray(
                    tuple(alloc.tensor_shape), mybir.dt.np(alloc.dtype)))
        self.in_dtypes = in_dtypes
        assert set(in_names) <= {"x", "shm", "sel", "shd"}, in_names
        assert out_names == ["out"], out_names
        all_in_names = in_names + out_names
        if partition_name is not None:
            all_in_names = all_in_names + [partition_name]
        self.in_names = in_names

        def _body(*args):
            operands = list(args)
            if partition_name is not None:
                operands.append(bass2jax.partition_id_tensor())
            return tuple(bass2jax._bass_exec_p.bind(
                *operands,
                out_avals=tuple(out_avals),
                in_names=tuple(all_in_names),
                out_names=tuple(out_names),
                lowering_input_output_aliases=(),
                sim_require_finite=True,
                sim_require_nnan=True,
                nc=nc,
            ))

        devices = jax.devices()[:N_CORES]
        assert len(devices) == N_CORES
        self.mesh = Mesh(np.asarray(devices), ("core",))
        spec = PartitionSpec("core")
        rep = PartitionSpec()
        self.sharding = NamedSharding(self.mesh, spec)
        in_specs = tuple(spec if n == "x" else rep for n in in_names) + (spec,)
        self.jitted = jax.jit(shard_map(
            _body, mesh=self.mesh, in_specs=in_specs, out_specs=(spec,),
            check_rep=False))
        # device-resident constant operands, created once
        self.zeros_dev = jax.device_put(
            np.zeros((N_CORES * LB, Z, A, K, NCLS), np.float32), self.sharding)
        consts = {}
        if "shm" in in_names:
            shm, sel, shd = _host_shift_mats()
            rep_sh = NamedSharding(self.mesh, rep)
            for n, arr in (("shm", shm), ("sel", sel), ("shd", shd)):
                if n in in_names:
                    consts[n] = jax.device_put(
                        arr.astype(in_dtypes[n]), rep_sh)
        self.consts = consts
        self._jax = jax

    def put(self, x: np.ndarray):
        return self._jax.device_put(
            np.ascontiguousarray(np.asarray(x, np.float32)), self.sharding)

    def run_dev(self, x_dev):
        """Execute; returns device array (not fetched)."""
        args = [x_dev if n == "x" else self.consts[n] for n in self.in_names]
        return self.jitted(*args, self.zeros_dev)[0]

    def __call__(self, x: np.ndarray) -> np.ndarray:
        return np.asarray(self.run_dev(self.put(x)))


_RUNNER = None


def _get_runner():
    global _RUNNER
    if _RUNNER is None:
        _RUNNER = _Runner()
    return _RUNNER


def kernel(x: np.ndarray) -> np.ndarray:
    x = np.asarray(x, dtype=np.float32)
    assert x.shape == (B, Z, A, C), x.shape
    try:
        return _get_runner()(x)
    except Exception:
        # fallback: reference-quality but slower dispatch path
        nc = _build()
        extra = {}
        if PE_SHIFT:
            shm, sel, shd = _host_shift_mats()
            cdt = mybir.dt.np(BF16)  # matches _build(planes=True) default
            extra = {"shm": shm.astype(cdt), "sel": sel.astype(cdt),
                     "shd": shd}
        in_maps = [{"x": np.ascontiguousarray(x[i * LB:(i + 1) * LB]), **extra}
                   for i in range(N_CORES)]
        res = run_bass_kernel_spmd(nc, in_maps, list(range(N_CORES)))
        return np.concatenate(
            [res.results[i]["out"] for i in range(N_CORES)], axis=0)

